# revision 46
# baseline (speedup 1.0000x reference)
"""AttentionSubsample Trainium2 kernel: 8-core data-parallel over batch.

Layout strategy (per core, 4 batch elements):
  - All matmuls contract over the SBUF partition dim.
  - k (fp32r matmul) and q (bf16 matmul) are evicted to bf16 with the BN
    shift added per-partition; scores then run as bf16 matmuls at full PE
    rate regardless of the 196-wide moving dim.
  - v computed token-major fp32r, evicted to bf16 [tok, head-blocks of
    64 dims + a ones column] so attn@v needs no transpose and yields the
    softmax denominator as output row 64 for free.
  - exp'd scores are written bf16 and multiplied by a host-gathered bf16
    exp-bias table (exp(s)*exp(bias)); the multiply is split between the
    DVE (2x bf16 mode) and the otherwise-idle Pool engine.
  - the bias table is loaded once per (head-pair, group) and reused across
    all 4 batch elements (b is the innermost loop of the attention phase).
  - denominator rows are staged at 32-aligned partitions (the partition
    crossbar only allows 32-aligned shifts), DMA-gathered, reciprocal'd to
    bf16, and broadcast back over 64-row blocks with a tiny bf16 matmul.
  - hardswish runs on the Pool engine (SBUF-only) in bf16; the projection
    is fp32r with the BN shift injected via a K=1 ones-row matmul.
"""

import sys

sys.path.insert(0, "/opt/trn_rl_repo")

from contextlib import ExitStack

import numpy as np
import ml_dtypes

import concourse.bass as bass
import concourse.tile as tile
from concourse import bacc
from concourse import mybir
from concourse.bass_utils import run_bass_kernel_spmd

F32 = mybir.dt.float32
F32R = mybir.dt.float32r
BF16 = mybir.dt.bfloat16
ALU = mybir.AluOpType
AF = mybir.ActivationFunctionType

B, N, NQ, IN, H, KD, D, OUT = 32, 784, 196, 384, 16, 32, 64, 512
HID, DH = 1536, 1024
RES, RES_, STRIDE = 28, 14, 2
SCALE = KD ** -0.5
EPS = 1e-5
NCORES = 8
BC = B // NCORES          # 4 batch elems per core
C, MC = 7, 112            # key-token chunks: 7 x 112 = 784
G, HG = 2, 8              # 2 head-groups of 8 heads

TRACE = False
LAST_RESULTS = None

_NC_CACHE = None


def _build_nc():
    nc = bacc.Bacc("TRN2", target_bir_lowering=False, debug=False,
                   num_devices=NCORES)

    xT = nc.dram_tensor("xT", [BC, IN, N], F32R, kind="ExternalInput").ap()
    xsT = nc.dram_tensor("xsT", [BC, IN, NQ], BF16, kind="ExternalInput").ap()
    wk = nc.dram_tensor("wk", [IN, 512], F32R, kind="ExternalInput").ap()
    wv = nc.dram_tensor("wv", [IN, DH], F32R, kind="ExternalInput").ap()
    wq = nc.dram_tensor("wq", [IN, 512], BF16, kind="ExternalInput").ap()
    wp = nc.dram_tensor("wp", [DH, OUT], BF16, kind="ExternalInput").ap()
    shk = nc.dram_tensor("shk", [128, 4], F32, kind="ExternalInput").ap()
    shq = nc.dram_tensor("shq", [128, 4], F32, kind="ExternalInput").ap()
    shv = nc.dram_tensor("shv", [128, 8], F32, kind="ExternalInput").ap()
    shp = nc.dram_tensor("shp", [1, OUT], F32R, kind="ExternalInput").ap()
    ebias = nc.dram_tensor("ebias", [MC, H, C, NQ], BF16,
                           kind="ExternalInput").ap()
    seld = nc.dram_tensor("seld", [16, 8, 128], BF16, kind="ExternalInput").ap()
    out = nc.dram_tensor("out", [BC, NQ, OUT], F32, kind="ExternalOutput").ap()




    with tile.TileContext(nc) as tc, ExitStack() as ctx:
        ctx.enter_context(nc.allow_low_precision(
            reason="bf16 attention path validated against fp32 reference"))
        singles = ctx.enter_context(tc.tile_pool(name="singles", bufs=1))
        biasp = ctx.enter_context(tc.tile_pool(name="biasp", bufs=3))
        xp = ctx.enter_context(tc.tile_pool(name="xp", bufs=3))
        texpp = ctx.enter_context(tc.tile_pool(name="texpp", bufs=8))
        tmpp = ctx.enter_context(tc.tile_pool(name="tmpp", bufs=3))
        hswp = ctx.enter_context(tc.tile_pool(name="hswp", bufs=2))
        finp = ctx.enter_context(tc.tile_pool(name="finp", bufs=3))
        mmp = ctx.enter_context(tc.tile_pool(name="mmp", bufs=2, space="PSUM"))
        scp = ctx.enter_context(tc.tile_pool(name="scp", bufs=2, space="PSUM"))
        opp = ctx.enter_context(tc.tile_pool(name="opp", bufs=2, space="PSUM"))

        # --- persistent SBUF ---
        wk_sb = singles.tile([128, 3, 512], F32R)
        nc.sync.dma_start(wk_sb, wk.rearrange("(c p) n -> p c n", p=128))
        wq_sb = singles.tile([128, 3, 512], BF16)
        nc.sync.dma_start(wq_sb, wq.rearrange("(c p) n -> p c n", p=128))
        wv_sb = singles.tile([128, 3, DH], F32R)
        nc.sync.dma_start(wv_sb, wv.rearrange("(c p) n -> p c n", p=128))
        wp_sb = singles.tile([128, 8, OUT], BF16)
        shk_sb = singles.tile([128, 4], F32)
        nc.sync.dma_start(shk_sb, shk)
        shq_sb = singles.tile([128, 4], F32)
        nc.sync.dma_start(shq_sb, shq)
        shv_sb = singles.tile([128, 8], F32)
        nc.sync.dma_start(shv_sb, shv)
        shp_sb = singles.tile([1, OUT], F32R)
        # sel[:, t, :] is a [16, 128] 0/1 matrix: sel[i, t, m] = 1 iff head i
        # feeds output row m of feature-tile t (rows 0-63 <- head 2t, 64-127
        # <- head 2t+1). Used to broadcast softmax reciprocals across rows.
        sel = singles.tile([16, 8, 128], BF16)
        nc.sync.dma_start(sel, seld)
        ones1 = singles.tile([1, 128], F32)
        nc.gpsimd.memset(ones1, 1.0)

        acc = [singles.tile([128, 8, NQ], BF16, name=f"acc{b}")
               for b in range(BC)]
        # denominator staging: head h=4g+hh -> partition 32*hh, block g
        den = [singles.tile([128, 4, NQ], F32, name=f"den{b}")
               for b in range(BC)]
        den2 = [singles.tile([16, NQ], F32, name=f"den2{b}") for b in range(BC)]
        recs = [singles.tile([16, NQ], BF16, name=f"rec{b}") for b in range(BC)]

        # per-b working tensors for the current head group
        kg = [singles.tile([128, 2, N], BF16, name=f"kg{b}") for b in range(BC)]
        qg = [singles.tile([128, 2, NQ], BF16, name=f"qg{b}")
              for b in range(BC)]
        vt = [singles.tile([MC, C, 8 * 65], BF16, name=f"vt{b}")
              for b in range(BC)]

        pending = None

        def emit_attn(texp2, hhs, b, g):
            # j=1 first: its bias product runs on the faster DVE path, so it
            # is ready sooner; j=0 (Pool) gets extra slack.
            for j, hh in ((1, hhs[1]), (0, hhs[0])):
                h = 8 * g + hh
                op = opp.tile([65, 256], F32, tag="op", name="op")
                for c in range(C):
                    nc.tensor.matmul(op[:, 0:196],
                                     lhsT=vt[b][:, c, 65 * hh:65 * hh + 65],
                                     rhs=texp2[j][:, c, :],
                                     start=(c == 0), stop=(c == C - 1))
                t = h // 2
                r0 = 64 * (h % 2)
                nc.vector.tensor_copy(acc[b][r0:r0 + 64, t, :],
                                      op[0:64, 0:196])
                nc.vector.tensor_copy(
                    den[b][32 * (h // 4):32 * (h // 4) + 1, h % 4, :],
                    op[64:65, 0:196])

        def emit_output(b):
            nc.sync.dma_start(
                den2[b],
                den[b].rearrange("(a c) d e -> a c d e", c=32)[:, 0, :, :])
            nc.vector.reciprocal(recs[b], den2[b])
            hsw = hswp.tile([128, 8, NQ], BF16, tag="hsw", name="hsw")
            for t in range(8):
                rep = mmp.tile([128, 512], F32, tag="mm", name="rep")
                if t < 4:
                    # K=8 slice (heads 0-7 cover all nonzero sel rows here);
                    # also probes small-K matmul behavior
                    nc.tensor.matmul(rep[:, :NQ], lhsT=sel[0:8, t, :],
                                     rhs=recs[b][0:8, :],
                                     start=True, stop=True)
                else:
                    nc.tensor.matmul(rep[:, :NQ], lhsT=sel[:, t, :],
                                     rhs=recs[b], start=True, stop=True)
                t1 = tmpp.tile([128, NQ], BF16, tag="t1", name="t1")
                nc.vector.tensor_tensor(t1, acc[b][:, t, :], rep[:, :NQ],
                                        ALU.mult)
                vv = tmpp.tile([128, NQ], BF16, tag="vv", name="vv")
                nc.scalar.activation(vv, t1, AF.Identity,
                                     bias=shv_sb[:, t:t + 1])
                t3 = tmpp.tile([128, NQ], BF16, tag="t3", name="t3")
                nc.vector.tensor_scalar(t3, vv, -3.0, 3.0, ALU.max, ALU.min)
                nc.vector.scalar_tensor_tensor(hsw[:, t, :], t3, 3.0, vv,
                                               ALU.add, ALU.mult)
            for mt, msz in ((0, 128), (1, 68)):
                po = mmp.tile([128, 512], F32, tag="mm", name="po")
                nc.tensor.matmul(po[:msz, :],
                                 lhsT=ones1.bitcast(F32R)[0:1, 0:msz],
                                 rhs=shp_sb, start=True, stop=False,
                                 skip_group_check=True)
                for kk in range(8):
                    nc.tensor.matmul(
                        po[:msz, :],
                        lhsT=hsw[:, kk, 128 * mt:128 * mt + msz],
                        rhs=wp_sb[:, kk, :], start=False,
                        stop=(kk == 7), skip_group_check=True)
                fin = finp.tile([128, OUT], F32, tag="fin", name="fin")
                nc.scalar.activation(fin[:msz, :], po[:msz, :], AF.Copy)
                nc.sync.dma_start(out[b, 128 * mt:128 * mt + msz, :],
                                  fin[:msz, :])

        def emit_phaseA(g, b):
            # ---- phase A: k, q, v for one (group, batch elem) ----
            xtb = xp.tile([128, 3, N], F32R, tag="xtb", name="xtb")
            nc.sync.dma_start(xtb, xT[b].rearrange("(c p) n -> p c n", p=128))
            xstb = xp.tile([128, 3, NQ], BF16, tag="xstb", name="xstb")
            nc.sync.dma_start(xstb, xsT[b].rearrange("(c p) n -> p c n", p=128))

            # k for this head group: features [256g, 256g+256), feat-major
            for m2 in range(2):
                for n2 in range(2):
                    pk = mmp.tile([128, 512], F32, tag="mm", name="pk")
                    for kk in range(3):
                        nc.tensor.matmul(
                            pk[:, :392],
                            lhsT=wk_sb[:, kk, 256 * g + 128 * m2:
                                       256 * g + 128 * m2 + 128],
                            rhs=xtb[:, kk, 392 * n2:392 * n2 + 392],
                            start=(kk == 0), stop=(kk == 2))
                    nc.vector.tensor_scalar_add(
                        kg[b][:, m2, 392 * n2:392 * n2 + 392],
                        pk[:, :392],
                        shk_sb[:, 2 * g + m2:2 * g + m2 + 1])

            # q for this head group (bf16 matmul)
            for m2 in range(2):
                pq = mmp.tile([128, 512], F32, tag="mm", name="pq")
                for kk in range(3):
                    nc.tensor.matmul(
                        pq[:, :NQ],
                        lhsT=wq_sb[:, kk, 256 * g + 128 * m2:
                                   256 * g + 128 * m2 + 128],
                        rhs=xstb[:, kk, :],
                        start=(kk == 0), stop=(kk == 2))
                nc.vector.tensor_scalar_add(
                    qg[b][:, m2, :], pq[:, :NQ],
                    shq_sb[:, 2 * g + m2:2 * g + m2 + 1])

            # v token-major for this head group (512 features), with an
            # all-ones column appended per head for the softmax denominator
            if g == 0:
                ones_cols = vt[b].rearrange(
                    "p c (h e) -> p c h e", e=65)[:, :, :, 64:65]
                nc.vector.memset(ones_cols, 1.0)
            for c in range(C):
                pv = mmp.tile([128, 512], F32, tag="mm", name="pv")
                for kk in range(3):
                    nc.tensor.matmul(
                        pv[:MC, :],
                        lhsT=xtb[:, kk, MC * c:MC * c + MC],
                        rhs=wv_sb[:, kk, 512 * g:512 * g + 512],
                        start=(kk == 0), stop=(kk == 2))
                nc.vector.tensor_copy(
                    vt[b].rearrange("p c (h e) -> p c h e", e=65)[:, c, :, 0:64],
                    pv[:MC, :].rearrange("p (h d) -> p h d", d=64))

        def emit_late_weights():
            nc.sync.dma_start(wp_sb, wp.rearrange("(c p) n -> p c n", p=128))
            nc.sync.dma_start(shp_sb, shp)

        emitted_A = set()
        for g in range(G):
            # flush the cross-group pending attn before phase A overwrites
            # the vt tiles it reads
            if pending is not None:
                pb_, phh, pbatch, pg = pending
                emit_attn(pb_, phh, pbatch, pg)
                if pg == 0 and phh[1] == HG - 1 and (1, pbatch) not in emitted_A:
                    emit_phaseA(1, pbatch)
                    emitted_A.add((1, pbatch))
                pending = None
            for b in range(BC):
                if (g, b) not in emitted_A:
                    emit_phaseA(g, b)
                    emitted_A.add((g, b))

            # ---- phase B: scores + attn, bias reused across batch ----
            for hp in range(HG // 2):
                hhs = (2 * hp, 2 * hp + 1)
                bias_g = biasp.tile([MC, 2, C, NQ], BF16, tag="bias")
                nc.sync.dma_start(
                    bias_g,
                    ebias[:, 8 * g + 2 * hp:8 * g + 2 * hp + 2, :, :])
                for b in range(BC):
                    texp2 = [texpp.tile([MC, C, NQ], BF16, tag="texp",
                                        name=f"texp{j}") for j in range(2)]
                    # scores in 2-bank tiles (4 chunks packed as 2x392);
                    # exps batched per cq half to amortize the Act engine's
                    # fixed per-instruction cost (Act paces this phase)
                    for cq, cs in ((0, (0, 1, 2, 3)), (1, (4, 5, 6))):
                        for j, hh in enumerate(hhs):
                            pb = 32 * (hh % 4)
                            m2 = hh // 4
                            sc = scp.tile([MC, 2, 512], F32, tag="sc",
                                          name="sc")
                            for ci, c in enumerate(cs):
                                nc.tensor.matmul(
                                    sc[:, ci // 2,
                                       196 * (ci % 2):196 * (ci % 2) + 196],
                                    lhsT=kg[b][pb:pb + 32, m2, MC * c:MC * c + MC],
                                    rhs=qg[b][pb:pb + 32, m2, :],
                                    start=True, stop=True,
                                    tile_position=(pb, 0),
                                    skip_group_check=True)
                            eng = nc.gpsimd if j == 0 else nc.vector
                            if cq == 0:
                                nc.scalar.activation(
                                    texp2[j][:, 0:4, :].rearrange(
                                        "p (a b) q -> p a b q", b=2),
                                    sc[:, :, 0:392].rearrange(
                                        "p a (b q) -> p a b q", q=196), AF.Exp)
                                eng.tensor_tensor(
                                    texp2[j][:, 0:4, :], texp2[j][:, 0:4, :],
                                    bias_g[:, j, 0:4, :], ALU.mult)
                            else:
                                nc.scalar.activation(
                                    texp2[j][:, 4:6, :],
                                    sc[:, 0, 0:392].rearrange(
                                        "p (a q) -> p a q", q=196), AF.Exp)
                                nc.scalar.activation(texp2[j][:, 6, :],
                                                     sc[:, 1, 0:196], AF.Exp)
                                eng.tensor_tensor(
                                    texp2[j][:, 4:7, :], texp2[j][:, 4:7, :],
                                    bias_g[:, j, 4:7, :], ALU.mult)

                    # attn @ v for the PREVIOUS iteration: by the time PE
                    # reaches these matmuls, that iteration's exp+bias chain
                    # has had a full iteration of slack, so PE never stalls
                    # on texp readiness.
                    if pending is not None:
                        pb_, phh, pbatch, pg = pending
                        emit_attn(pb_, phh, pbatch, pg)
                        if phh[1] == HG - 1:
                            if pg == G - 1:
                                # last head pair of pbatch done: its output
                                # projection interleaves with remaining
                                # attention iterations
                                emit_output(pbatch)
                            elif (1, pbatch) not in emitted_A:
                                # pbatch finished group 0: its group-1 k/q/v
                                # compute and the first-half output chain
                                # interleave with remaining group-0 attention
                                emit_phaseA(1, pbatch)
                                emitted_A.add((1, pbatch))
                                if pbatch == 0:
                                    emit_late_weights()
                    pending = (texp2, hhs, b, g)

            if g == G - 1:
                pb_, phh, pbatch, pg = pending
                emit_attn(pb_, phh, pbatch, pg)
                emit_output(pbatch)
                pending = None
    nc.compile()
    return nc


def _prepare_in_maps(inputs):
    inp = {k: np.asarray(v) for k, v in inputs.items()}
    x = inp["x"].astype(np.float32)          # [32, 784, 384]
    Wkv, Wq, Wp = inp["Wkv"], inp["Wq"], inp["Wp"]
    biases, idxs = inp["biases"], inp["idxs"].astype(np.int64)

    s_kv = inp["kv_w"] / np.sqrt(inp["kv_var"] + EPS)
    wkv = (Wkv * s_kv[:, None]).astype(np.float32)
    sh_kv = (inp["kv_b"] - inp["kv_mean"] * s_kv).astype(np.float32)
    wkv3 = wkv.reshape(H, KD + D, IN)
    sh3 = sh_kv.reshape(H, KD + D)
    wkT = np.ascontiguousarray(wkv3[:, :KD, :].reshape(H * KD, IN).T)
    sh_k = np.ascontiguousarray(sh3[:, :KD].reshape(H * KD))
    wvT = np.ascontiguousarray(wkv3[:, KD:, :].reshape(H * D, IN).T)
    sh_v = np.ascontiguousarray(sh3[:, KD:].reshape(H * D))

    s_q = inp["q_w"] / np.sqrt(inp["q_var"] + EPS)
    wqT = np.ascontiguousarray(
        (Wq * (s_q * SCALE)[:, None]).T.astype(ml_dtypes.bfloat16))
    sh_q = ((inp["q_b"] - inp["q_mean"] * s_q) * SCALE).astype(np.float32)

    s_p = inp["p_w"] / np.sqrt(inp["p_var"] + EPS)
    wpT = np.ascontiguousarray(
        ((Wp * s_p[:, None]) / 6.0).T.astype(ml_dtypes.bfloat16))
    sh_p = (inp["p_b"] - inp["p_mean"] * s_p).astype(np.float32)

    eb = np.exp(biases.astype(np.float64))[:, idxs]      # [16, 196, 784]
    eb = eb.transpose(0, 2, 1).reshape(H, C, MC, NQ)
    eb = np.ascontiguousarray(eb.transpose(2, 0, 1, 3)).astype(ml_dtypes.bfloat16)

    xs = x.reshape(B, RES, RES, IN)[:, ::STRIDE, ::STRIDE].reshape(B, NQ, IN)

    shk_h = np.ascontiguousarray(sh_k.reshape(4, 128).T)
    shq_h = np.ascontiguousarray(sh_q.reshape(4, 128).T)
    shv_h = np.ascontiguousarray(sh_v.reshape(8, 128).T)
    shp_h = np.ascontiguousarray(sh_p.reshape(1, OUT))

    sel_h = np.zeros((16, 8, 128), ml_dtypes.bfloat16)
    for t in range(8):
        sel_h[2 * t, t, 0:64] = 1.0
        sel_h[2 * t + 1, t, 64:128] = 1.0
    shared = {"wk": wkT, "wv": wvT, "wq": wqT, "wp": wpT, "shk": shk_h,
              "shq": shq_h, "shv": shv_h, "shp": shp_h, "ebias": eb,
              "seld": sel_h}
    in_maps = []
    for i in range(NCORES):
        xb = x[BC * i:BC * i + BC]
        xsb = xs[BC * i:BC * i + BC]
        m = dict(shared)
        m["xT"] = np.ascontiguousarray(xb.transpose(0, 2, 1))
        m["xsT"] = np.ascontiguousarray(
            xsb.transpose(0, 2, 1).astype(ml_dtypes.bfloat16))
        in_maps.append(m)
    return in_maps


def kernel(**inputs):
    global _NC_CACHE, LAST_RESULTS
    in_maps = _prepare_in_maps(inputs)
    if _NC_CACHE is None:
        _NC_CACHE = _build_nc()
    res = run_bass_kernel_spmd(_NC_CACHE, in_maps,
                               core_ids=list(range(NCORES)), trace=TRACE)
    LAST_RESULTS = res
    return np.concatenate([res.results[i]["out"] for i in range(NCORES)],
                          axis=0)


# revision 47
# speedup vs baseline: 1.0014x; 1.0014x over previous
"""AttentionSubsample Trainium2 kernel: 8-core data-parallel over batch.

Layout strategy (per core, 4 batch elements):
  - All matmuls contract over the SBUF partition dim.
  - k (fp32r matmul) and q (bf16 matmul) are evicted to bf16 with the BN
    shift added per-partition; scores then run as bf16 matmuls at full PE
    rate regardless of the 196-wide moving dim.
  - v computed token-major fp32r, evicted to bf16 [tok, head-blocks of
    64 dims + a ones column] so attn@v needs no transpose and yields the
    softmax denominator as output row 64 for free.
  - exp'd scores are written bf16 and multiplied by a host-gathered bf16
    exp-bias table (exp(s)*exp(bias)); the multiply is split between the
    DVE (2x bf16 mode) and the otherwise-idle Pool engine.
  - the bias table is loaded once per (head-pair, group) and reused across
    all 4 batch elements (b is the innermost loop of the attention phase).
  - denominator rows are staged at 32-aligned partitions (the partition
    crossbar only allows 32-aligned shifts), DMA-gathered, reciprocal'd to
    bf16, and broadcast back over 64-row blocks with a tiny bf16 matmul.
  - hardswish runs on the Pool engine (SBUF-only) in bf16; the projection
    is fp32r with the BN shift injected via a K=1 ones-row matmul.
"""

import sys

sys.path.insert(0, "/opt/trn_rl_repo")

from contextlib import ExitStack

import numpy as np
import ml_dtypes

import concourse.bass as bass
import concourse.tile as tile
from concourse import bacc
from concourse import mybir
from concourse.bass_utils import run_bass_kernel_spmd

F32 = mybir.dt.float32
F32R = mybir.dt.float32r
BF16 = mybir.dt.bfloat16
ALU = mybir.AluOpType
AF = mybir.ActivationFunctionType

B, N, NQ, IN, H, KD, D, OUT = 32, 784, 196, 384, 16, 32, 64, 512
HID, DH = 1536, 1024
RES, RES_, STRIDE = 28, 14, 2
SCALE = KD ** -0.5
EPS = 1e-5
NCORES = 8
BC = B // NCORES          # 4 batch elems per core
C, MC = 7, 112            # key-token chunks: 7 x 112 = 784
G, HG = 2, 8              # 2 head-groups of 8 heads

TRACE = False
LAST_RESULTS = None

_NC_CACHE = None


def _build_nc():
    nc = bacc.Bacc("TRN2", target_bir_lowering=False, debug=False,
                   num_devices=NCORES)

    xT = nc.dram_tensor("xT", [BC, IN, N], F32R, kind="ExternalInput").ap()
    xsT = nc.dram_tensor("xsT", [BC, IN, NQ], BF16, kind="ExternalInput").ap()
    wk = nc.dram_tensor("wk", [IN, 512], F32R, kind="ExternalInput").ap()
    wv = nc.dram_tensor("wv", [IN, DH], F32R, kind="ExternalInput").ap()
    wq = nc.dram_tensor("wq", [IN, 512], BF16, kind="ExternalInput").ap()
    wp = nc.dram_tensor("wp", [DH, OUT], BF16, kind="ExternalInput").ap()
    shk = nc.dram_tensor("shk", [128, 4], F32, kind="ExternalInput").ap()
    shq = nc.dram_tensor("shq", [128, 4], F32, kind="ExternalInput").ap()
    shv = nc.dram_tensor("shv", [128, 8], F32, kind="ExternalInput").ap()
    shp = nc.dram_tensor("shp", [1, OUT], F32R, kind="ExternalInput").ap()
    ebias = nc.dram_tensor("ebias", [MC, H, C, NQ], BF16,
                           kind="ExternalInput").ap()
    seld = nc.dram_tensor("seld", [16, 8, 128], BF16, kind="ExternalInput").ap()
    out = nc.dram_tensor("out", [BC, NQ, OUT], F32, kind="ExternalOutput").ap()




    with tile.TileContext(nc) as tc, ExitStack() as ctx:
        ctx.enter_context(nc.allow_low_precision(
            reason="bf16 attention path validated against fp32 reference"))
        singles = ctx.enter_context(tc.tile_pool(name="singles", bufs=1))
        biasp = ctx.enter_context(tc.tile_pool(name="biasp", bufs=3))
        xp = ctx.enter_context(tc.tile_pool(name="xp", bufs=3))
        texpp = ctx.enter_context(tc.tile_pool(name="texpp", bufs=8))
        tmpp = ctx.enter_context(tc.tile_pool(name="tmpp", bufs=2))
        hswp = ctx.enter_context(tc.tile_pool(name="hswp", bufs=2))
        finp = ctx.enter_context(tc.tile_pool(name="finp", bufs=2))
        mmp = ctx.enter_context(tc.tile_pool(name="mmp", bufs=2, space="PSUM"))
        scp = ctx.enter_context(tc.tile_pool(name="scp", bufs=2, space="PSUM"))
        opp = ctx.enter_context(tc.tile_pool(name="opp", bufs=2, space="PSUM"))

        # --- persistent SBUF ---
        wk_sb = singles.tile([128, 3, 512], F32R)
        nc.sync.dma_start(wk_sb, wk.rearrange("(c p) n -> p c n", p=128))
        wq_sb = singles.tile([128, 3, 512], BF16)
        nc.sync.dma_start(wq_sb, wq.rearrange("(c p) n -> p c n", p=128))
        wv_sb = singles.tile([128, 3, DH], F32R)
        nc.sync.dma_start(wv_sb, wv.rearrange("(c p) n -> p c n", p=128))
        wp_sb = singles.tile([128, 8, OUT], BF16)
        shk_sb = singles.tile([128, 4], F32)
        nc.sync.dma_start(shk_sb, shk)
        shq_sb = singles.tile([128, 4], F32)
        nc.sync.dma_start(shq_sb, shq)
        shv_sb = singles.tile([128, 8], F32)
        nc.sync.dma_start(shv_sb, shv)
        shp_sb = singles.tile([1, OUT], F32R)
        # sel[:, t, :] is a [16, 128] 0/1 matrix: sel[i, t, m] = 1 iff head i
        # feeds output row m of feature-tile t (rows 0-63 <- head 2t, 64-127
        # <- head 2t+1). Used to broadcast softmax reciprocals across rows.
        sel = singles.tile([16, 8, 128], BF16)
        nc.sync.dma_start(sel, seld)
        ones1 = singles.tile([1, 128], F32)
        nc.gpsimd.memset(ones1, 1.0)

        acc = [singles.tile([128, 8, NQ], BF16, name=f"acc{b}")
               for b in range(BC)]
        # denominator staging: head h=4g+hh -> partition 32*hh, block g
        den = [singles.tile([128, 4, NQ], F32, name=f"den{b}")
               for b in range(BC)]
        den2 = [singles.tile([16, NQ], F32, name=f"den2{b}") for b in range(BC)]
        recs = [singles.tile([16, NQ], BF16, name=f"rec{b}") for b in range(BC)]

        # per-b working tensors for the current head group
        kg = [singles.tile([128, 2, N], BF16, name=f"kg{b}") for b in range(BC)]
        qg = [singles.tile([128, 2, NQ], BF16, name=f"qg{b}")
              for b in range(BC)]
        vt = [singles.tile([MC, C, 8 * 65], BF16, name=f"vt{b}")
              for b in range(BC)]

        pending = None

        def emit_attn(texp2, hhs, b, g):
            # j=1 first: its bias product runs on the faster DVE path, so it
            # is ready sooner; j=0 (Pool) gets extra slack.
            for j, hh in ((1, hhs[1]), (0, hhs[0])):
                h = 8 * g + hh
                op = opp.tile([65, 256], F32, tag="op", name="op")
                for c in range(C):
                    nc.tensor.matmul(op[:, 0:196],
                                     lhsT=vt[b][:, c, 65 * hh:65 * hh + 65],
                                     rhs=texp2[j][:, c, :],
                                     start=(c == 0), stop=(c == C - 1))
                t = h // 2
                r0 = 64 * (h % 2)
                nc.vector.tensor_copy(acc[b][r0:r0 + 64, t, :],
                                      op[0:64, 0:196])
                nc.vector.tensor_copy(
                    den[b][32 * (h // 4):32 * (h // 4) + 1, h % 4, :],
                    op[64:65, 0:196])

        def emit_output(b):
            nc.sync.dma_start(
                den2[b],
                den[b].rearrange("(a c) d e -> a c d e", c=32)[:, 0, :, :])
            nc.vector.reciprocal(recs[b], den2[b])
            hsw = hswp.tile([128, 8, NQ], BF16, tag="hsw", name="hsw")
            for t in range(8):
                rep = mmp.tile([128, 512], F32, tag="mm", name="rep")
                if t < 4:
                    # K=8 slice (heads 0-7 cover all nonzero sel rows here);
                    # also probes small-K matmul behavior
                    nc.tensor.matmul(rep[:, :NQ], lhsT=sel[0:8, t, :],
                                     rhs=recs[b][0:8, :],
                                     start=True, stop=True)
                else:
                    nc.tensor.matmul(rep[:, :NQ], lhsT=sel[:, t, :],
                                     rhs=recs[b], start=True, stop=True)
                t1 = tmpp.tile([128, NQ], BF16, tag="t1", name="t1")
                nc.vector.tensor_tensor(t1, acc[b][:, t, :], rep[:, :NQ],
                                        ALU.mult)
                vv = tmpp.tile([128, NQ], BF16, tag="vv", name="vv")
                nc.scalar.activation(vv, t1, AF.Identity,
                                     bias=shv_sb[:, t:t + 1])
                t3 = tmpp.tile([128, NQ], BF16, tag="t3", name="t3")
                nc.vector.tensor_scalar(t3, vv, -3.0, 3.0, ALU.max, ALU.min)
                nc.vector.scalar_tensor_tensor(hsw[:, t, :], t3, 3.0, vv,
                                               ALU.add, ALU.mult)
            for mt, msz in ((0, 128), (1, 68)):
                po = mmp.tile([128, 512], F32, tag="mm", name="po")
                nc.tensor.matmul(po[:msz, :],
                                 lhsT=ones1.bitcast(F32R)[0:1, 0:msz],
                                 rhs=shp_sb, start=True, stop=False,
                                 skip_group_check=True)
                for kk in range(8):
                    nc.tensor.matmul(
                        po[:msz, :],
                        lhsT=hsw[:, kk, 128 * mt:128 * mt + msz],
                        rhs=wp_sb[:, kk, :], start=False,
                        stop=(kk == 7), skip_group_check=True)
                fin = finp.tile([128, OUT], F32, tag="fin", name="fin")
                nc.scalar.activation(fin[:msz, :], po[:msz, :], AF.Copy)
                nc.sync.dma_start(out[b, 128 * mt:128 * mt + msz, :],
                                  fin[:msz, :])

        def emit_phaseA(g, b):
            # ---- phase A: k, q, v for one (group, batch elem) ----
            xtb = xp.tile([128, 3, N], F32R, tag="xtb", name="xtb")
            nc.sync.dma_start(xtb, xT[b].rearrange("(c p) n -> p c n", p=128))
            xstb = xp.tile([128, 3, NQ], BF16, tag="xstb", name="xstb")
            nc.sync.dma_start(xstb, xsT[b].rearrange("(c p) n -> p c n", p=128))

            # k for this head group: features [256g, 256g+256), feat-major
            for m2 in range(2):
                for n2 in range(2):
                    pk = mmp.tile([128, 512], F32, tag="mm", name="pk")
                    for kk in range(3):
                        nc.tensor.matmul(
                            pk[:, :392],
                            lhsT=wk_sb[:, kk, 256 * g + 128 * m2:
                                       256 * g + 128 * m2 + 128],
                            rhs=xtb[:, kk, 392 * n2:392 * n2 + 392],
                            start=(kk == 0), stop=(kk == 2))
                    nc.vector.tensor_scalar_add(
                        kg[b][:, m2, 392 * n2:392 * n2 + 392],
                        pk[:, :392],
                        shk_sb[:, 2 * g + m2:2 * g + m2 + 1])

            # q for this head group (bf16 matmul)
            for m2 in range(2):
                pq = mmp.tile([128, 512], F32, tag="mm", name="pq")
                for kk in range(3):
                    nc.tensor.matmul(
                        pq[:, :NQ],
                        lhsT=wq_sb[:, kk, 256 * g + 128 * m2:
                                   256 * g + 128 * m2 + 128],
                        rhs=xstb[:, kk, :],
                        start=(kk == 0), stop=(kk == 2))
                nc.vector.tensor_scalar_add(
                    qg[b][:, m2, :], pq[:, :NQ],
                    shq_sb[:, 2 * g + m2:2 * g + m2 + 1])

            # v token-major for this head group (512 features), with an
            # all-ones column appended per head for the softmax denominator
            if g == 0:
                ones_cols = vt[b].rearrange(
                    "p c (h e) -> p c h e", e=65)[:, :, :, 64:65]
                nc.vector.memset(ones_cols, 1.0)
            for c in range(C):
                pv = mmp.tile([128, 512], F32, tag="mm", name="pv")
                for kk in range(3):
                    nc.tensor.matmul(
                        pv[:MC, :],
                        lhsT=xtb[:, kk, MC * c:MC * c + MC],
                        rhs=wv_sb[:, kk, 512 * g:512 * g + 512],
                        start=(kk == 0), stop=(kk == 2))
                nc.vector.tensor_copy(
                    vt[b].rearrange("p c (h e) -> p c h e", e=65)[:, c, :, 0:64],
                    pv[:MC, :].rearrange("p (h d) -> p h d", d=64))

        def emit_late_weights():
            nc.sync.dma_start(wp_sb, wp.rearrange("(c p) n -> p c n", p=128))
            nc.sync.dma_start(shp_sb, shp)

        emitted_A = set()
        for g in range(G):
            # flush the cross-group pending attn before phase A overwrites
            # the vt tiles it reads
            if pending is not None:
                pb_, phh, pbatch, pg = pending
                emit_attn(pb_, phh, pbatch, pg)
                if pg == 0 and phh[1] == HG - 1 and (1, pbatch) not in emitted_A:
                    emit_phaseA(1, pbatch)
                    emitted_A.add((1, pbatch))
                pending = None
            for b in range(BC):
                if (g, b) not in emitted_A:
                    emit_phaseA(g, b)
                    emitted_A.add((g, b))

            # ---- phase B: scores + attn, bias reused across batch ----
            for hp in range(HG // 2):
                hhs = (2 * hp, 2 * hp + 1)
                bias_g = biasp.tile([MC, 2, C, NQ], BF16, tag="bias")
                nc.sync.dma_start(
                    bias_g,
                    ebias[:, 8 * g + 2 * hp:8 * g + 2 * hp + 2, :, :])
                for b in range(BC):
                    texp2 = [texpp.tile([MC, C, NQ], BF16, tag="texp",
                                        name=f"texp{j}") for j in range(2)]
                    # scores in 2-bank tiles (4 chunks packed as 2x392);
                    # exps batched per cq half to amortize the Act engine's
                    # fixed per-instruction cost (Act paces this phase)
                    for cq, cs in ((0, (0, 1, 2, 3)), (1, (4, 5, 6))):
                        for j, hh in enumerate(hhs):
                            pb = 32 * (hh % 4)
                            m2 = hh // 4
                            sc = scp.tile([MC, 2, 512], F32, tag="sc",
                                          name="sc")
                            for ci, c in enumerate(cs):
                                nc.tensor.matmul(
                                    sc[:, ci // 2,
                                       196 * (ci % 2):196 * (ci % 2) + 196],
                                    lhsT=kg[b][pb:pb + 32, m2, MC * c:MC * c + MC],
                                    rhs=qg[b][pb:pb + 32, m2, :],
                                    start=True, stop=True,
                                    tile_position=(pb, 0),
                                    skip_group_check=True)
                            eng = nc.gpsimd if j == 0 else nc.vector
                            if cq == 0:
                                nc.scalar.activation(
                                    texp2[j][:, 0:4, :].rearrange(
                                        "p (a b) q -> p a b q", b=2),
                                    sc[:, :, 0:392].rearrange(
                                        "p a (b q) -> p a b q", q=196), AF.Exp)
                                eng.tensor_tensor(
                                    texp2[j][:, 0:4, :], texp2[j][:, 0:4, :],
                                    bias_g[:, j, 0:4, :], ALU.mult)
                            else:
                                nc.scalar.activation(
                                    texp2[j][:, 4:6, :],
                                    sc[:, 0, 0:392].rearrange(
                                        "p (a q) -> p a q", q=196), AF.Exp)
                                nc.scalar.activation(texp2[j][:, 6, :],
                                                     sc[:, 1, 0:196], AF.Exp)
                                eng.tensor_tensor(
                                    texp2[j][:, 4:7, :], texp2[j][:, 4:7, :],
                                    bias_g[:, j, 4:7, :], ALU.mult)

                    # attn @ v for the PREVIOUS iteration: by the time PE
                    # reaches these matmuls, that iteration's exp+bias chain
                    # has had a full iteration of slack, so PE never stalls
                    # on texp readiness.
                    if pending is not None:
                        pb_, phh, pbatch, pg = pending
                        emit_attn(pb_, phh, pbatch, pg)
                        if phh[1] == HG - 1:
                            if pg == G - 1:
                                # last head pair of pbatch done: its output
                                # projection interleaves with remaining
                                # attention iterations
                                emit_output(pbatch)
                            elif (1, pbatch) not in emitted_A:
                                # pbatch finished group 0: its group-1 k/q/v
                                # compute and the first-half output chain
                                # interleave with remaining group-0 attention
                                emit_phaseA(1, pbatch)
                                emitted_A.add((1, pbatch))
                                if pbatch == 0:
                                    emit_late_weights()
                    pending = (texp2, hhs, b, g)

            if g == G - 1:
                pb_, phh, pbatch, pg = pending
                emit_attn(pb_, phh, pbatch, pg)
                emit_output(pbatch)
                pending = None
    nc.compile()
    return nc


def _prepare_in_maps(inputs):
    inp = {k: np.asarray(v) for k, v in inputs.items()}
    x = inp["x"].astype(np.float32)          # [32, 784, 384]
    Wkv, Wq, Wp = inp["Wkv"], inp["Wq"], inp["Wp"]
    biases, idxs = inp["biases"], inp["idxs"].astype(np.int64)

    s_kv = inp["kv_w"] / np.sqrt(inp["kv_var"] + EPS)
    wkv = (Wkv * s_kv[:, None]).astype(np.float32)
    sh_kv = (inp["kv_b"] - inp["kv_mean"] * s_kv).astype(np.float32)
    wkv3 = wkv.reshape(H, KD + D, IN)
    sh3 = sh_kv.reshape(H, KD + D)
    wkT = np.ascontiguousarray(wkv3[:, :KD, :].reshape(H * KD, IN).T)
    sh_k = np.ascontiguousarray(sh3[:, :KD].reshape(H * KD))
    wvT = np.ascontiguousarray(wkv3[:, KD:, :].reshape(H * D, IN).T)
    sh_v = np.ascontiguousarray(sh3[:, KD:].reshape(H * D))

    s_q = inp["q_w"] / np.sqrt(inp["q_var"] + EPS)
    wqT = np.ascontiguousarray(
        (Wq * (s_q * SCALE)[:, None]).T.astype(ml_dtypes.bfloat16))
    sh_q = ((inp["q_b"] - inp["q_mean"] * s_q) * SCALE).astype(np.float32)

    s_p = inp["p_w"] / np.sqrt(inp["p_var"] + EPS)
    wpT = np.ascontiguousarray(
        ((Wp * s_p[:, None]) / 6.0).T.astype(ml_dtypes.bfloat16))
    sh_p = (inp["p_b"] - inp["p_mean"] * s_p).astype(np.float32)

    eb = np.exp(biases.astype(np.float64))[:, idxs]      # [16, 196, 784]
    eb = eb.transpose(0, 2, 1).reshape(H, C, MC, NQ)
    eb = np.ascontiguousarray(eb.transpose(2, 0, 1, 3)).astype(ml_dtypes.bfloat16)

    xs = x.reshape(B, RES, RES, IN)[:, ::STRIDE, ::STRIDE].reshape(B, NQ, IN)

    shk_h = np.ascontiguousarray(sh_k.reshape(4, 128).T)
    shq_h = np.ascontiguousarray(sh_q.reshape(4, 128).T)
    shv_h = np.ascontiguousarray(sh_v.reshape(8, 128).T)
    shp_h = np.ascontiguousarray(sh_p.reshape(1, OUT))

    sel_h = np.zeros((16, 8, 128), ml_dtypes.bfloat16)
    for t in range(8):
        sel_h[2 * t, t, 0:64] = 1.0
        sel_h[2 * t + 1, t, 64:128] = 1.0
    shared = {"wk": wkT, "wv": wvT, "wq": wqT, "wp": wpT, "shk": shk_h,
              "shq": shq_h, "shv": shv_h, "shp": shp_h, "ebias": eb,
              "seld": sel_h}
    in_maps = []
    for i in range(NCORES):
        xb = x[BC * i:BC * i + BC]
        xsb = xs[BC * i:BC * i + BC]
        m = dict(shared)
        m["xT"] = np.ascontiguousarray(xb.transpose(0, 2, 1))
        m["xsT"] = np.ascontiguousarray(
            xsb.transpose(0, 2, 1).astype(ml_dtypes.bfloat16))
        in_maps.append(m)
    return in_maps


def kernel(**inputs):
    global _NC_CACHE, LAST_RESULTS
    in_maps = _prepare_in_maps(inputs)
    if _NC_CACHE is None:
        _NC_CACHE = _build_nc()
    res = run_bass_kernel_spmd(_NC_CACHE, in_maps,
                               core_ids=list(range(NCORES)), trace=TRACE)
    LAST_RESULTS = res
    return np.concatenate([res.results[i]["out"] for i in range(NCORES)],
                          axis=0)


# revision 48
# speedup vs baseline: 1.0267x; 1.0252x over previous
"""AttentionSubsample Trainium2 kernel: 8-core data-parallel over batch.

Layout strategy (per core, 4 batch elements):
  - All matmuls contract over the SBUF partition dim.
  - k (fp32r matmul) and q (bf16 matmul) are evicted to bf16 with the BN
    shift added per-partition; scores then run as bf16 matmuls at full PE
    rate regardless of the 196-wide moving dim.
  - v computed token-major fp32r, evicted to bf16 [tok, head-blocks of
    64 dims + a ones column] so attn@v needs no transpose and yields the
    softmax denominator as output row 64 for free.
  - exp'd scores are written bf16 and multiplied by a host-gathered bf16
    exp-bias table (exp(s)*exp(bias)); the multiply is split between the
    DVE (2x bf16 mode) and the otherwise-idle Pool engine.
  - the bias table is loaded once per (head-pair, group) and reused across
    all 4 batch elements (b is the innermost loop of the attention phase).
  - denominator rows are staged at 32-aligned partitions (the partition
    crossbar only allows 32-aligned shifts), DMA-gathered, reciprocal'd to
    bf16, and broadcast back over 64-row blocks with a tiny bf16 matmul.
  - hardswish runs on the Pool engine (SBUF-only) in bf16; the projection
    is fp32r with the BN shift injected via a K=1 ones-row matmul.
"""

import sys

sys.path.insert(0, "/opt/trn_rl_repo")

from contextlib import ExitStack

import numpy as np
import ml_dtypes

import concourse.bass as bass
import concourse.tile as tile
from concourse import bacc
from concourse import mybir
from concourse.bass_utils import run_bass_kernel_spmd

F32 = mybir.dt.float32
F32R = mybir.dt.float32r
BF16 = mybir.dt.bfloat16
ALU = mybir.AluOpType
AF = mybir.ActivationFunctionType

B, N, NQ, IN, H, KD, D, OUT = 32, 784, 196, 384, 16, 32, 64, 512
HID, DH = 1536, 1024
RES, RES_, STRIDE = 28, 14, 2
SCALE = KD ** -0.5
EPS = 1e-5
NCORES = 8
BC = B // NCORES          # 4 batch elems per core
C, MC = 7, 112            # key-token chunks: 7 x 112 = 784
G, HG = 2, 8              # 2 head-groups of 8 heads

TRACE = False
LAST_RESULTS = None

_NC_CACHE = None


def _build_nc():
    nc = bacc.Bacc("TRN2", target_bir_lowering=False, debug=False,
                   num_devices=NCORES)

    xT = nc.dram_tensor("xT", [BC, IN, N], F32R, kind="ExternalInput").ap()
    xsT = nc.dram_tensor("xsT", [BC, IN, NQ], BF16, kind="ExternalInput").ap()
    wk = nc.dram_tensor("wk", [IN, 512], F32R, kind="ExternalInput").ap()
    wv = nc.dram_tensor("wv", [IN, DH], F32R, kind="ExternalInput").ap()
    wq = nc.dram_tensor("wq", [IN, 512], BF16, kind="ExternalInput").ap()
    wp = nc.dram_tensor("wp", [DH, OUT], BF16, kind="ExternalInput").ap()
    shk = nc.dram_tensor("shk", [128, 4], F32, kind="ExternalInput").ap()
    shq = nc.dram_tensor("shq", [128, 4], F32, kind="ExternalInput").ap()
    shv = nc.dram_tensor("shv", [128, 8], F32, kind="ExternalInput").ap()
    shp = nc.dram_tensor("shp", [1, OUT], F32R, kind="ExternalInput").ap()
    ebias = nc.dram_tensor("ebias", [MC, H, C, NQ], BF16,
                           kind="ExternalInput").ap()
    seld = nc.dram_tensor("seld", [16, 8, 128], BF16, kind="ExternalInput").ap()
    out = nc.dram_tensor("out", [BC, NQ, OUT], F32, kind="ExternalOutput").ap()




    with tile.TileContext(nc) as tc, ExitStack() as ctx:
        ctx.enter_context(nc.allow_low_precision(
            reason="bf16 attention path validated against fp32 reference"))
        singles = ctx.enter_context(tc.tile_pool(name="singles", bufs=1))
        biasp = ctx.enter_context(tc.tile_pool(name="biasp", bufs=3))
        xp = ctx.enter_context(tc.tile_pool(name="xp", bufs=3))
        texpp = ctx.enter_context(tc.tile_pool(name="texpp", bufs=8))
        tmpp = ctx.enter_context(tc.tile_pool(name="tmpp", bufs=2))
        hswp = ctx.enter_context(tc.tile_pool(name="hswp", bufs=2))
        finp = ctx.enter_context(tc.tile_pool(name="finp", bufs=2))
        mmp = ctx.enter_context(tc.tile_pool(name="mmp", bufs=2, space="PSUM"))
        scp = ctx.enter_context(tc.tile_pool(name="scp", bufs=2, space="PSUM"))
        opp = ctx.enter_context(tc.tile_pool(name="opp", bufs=2, space="PSUM"))

        # --- persistent SBUF ---
        wk_sb = singles.tile([128, 3, 512], F32R)
        nc.sync.dma_start(wk_sb, wk.rearrange("(c p) n -> p c n", p=128))
        wq_sb = singles.tile([128, 3, 512], BF16)
        wv_sb = singles.tile([128, 3, DH], F32R)
        wp_sb = singles.tile([128, 8, OUT], BF16)
        shk_sb = singles.tile([128, 4], F32)
        nc.sync.dma_start(shk_sb, shk)
        shq_sb = singles.tile([128, 4], F32)
        nc.sync.dma_start(shq_sb, shq)
        shv_sb = singles.tile([128, 8], F32)
        nc.sync.dma_start(shv_sb, shv)
        shp_sb = singles.tile([1, OUT], F32R)
        # sel[:, t, :] is a [16, 128] 0/1 matrix: sel[i, t, m] = 1 iff head i
        # feeds output row m of feature-tile t (rows 0-63 <- head 2t, 64-127
        # <- head 2t+1). Used to broadcast softmax reciprocals across rows.
        sel = singles.tile([16, 8, 128], BF16)
        nc.sync.dma_start(sel, seld)
        ones1 = singles.tile([1, 128], F32)
        nc.gpsimd.memset(ones1, 1.0)

        acc = [singles.tile([128, 8, NQ], BF16, name=f"acc{b}")
               for b in range(BC)]
        # denominator staging: head h=4g+hh -> partition 32*hh, block g
        den = [singles.tile([128, 4, NQ], F32, name=f"den{b}")
               for b in range(BC)]
        den2 = [singles.tile([16, NQ], F32, name=f"den2{b}") for b in range(BC)]
        recs = [singles.tile([16, NQ], BF16, name=f"rec{b}") for b in range(BC)]

        # per-b working tensors for the current head group
        kg = [singles.tile([128, 2, N], BF16, name=f"kg{b}") for b in range(BC)]
        qg = [singles.tile([128, 2, NQ], BF16, name=f"qg{b}")
              for b in range(BC)]
        vt = [singles.tile([MC, C, 8 * 65], BF16, name=f"vt{b}")
              for b in range(BC)]

        pending = None

        def emit_attn(texp2, hhs, b, g):
            # j=1 first: its bias product runs on the faster DVE path, so it
            # is ready sooner; j=0 (Pool) gets extra slack.
            for j, hh in ((1, hhs[1]), (0, hhs[0])):
                h = 8 * g + hh
                op = opp.tile([65, 256], F32, tag="op", name="op")
                for c in range(C):
                    nc.tensor.matmul(op[:, 0:196],
                                     lhsT=vt[b][:, c, 65 * hh:65 * hh + 65],
                                     rhs=texp2[j][:, c, :],
                                     start=(c == 0), stop=(c == C - 1))
                t = h // 2
                r0 = 64 * (h % 2)
                nc.vector.tensor_copy(acc[b][r0:r0 + 64, t, :],
                                      op[0:64, 0:196])
                nc.vector.tensor_copy(
                    den[b][32 * (h // 4):32 * (h // 4) + 1, h % 4, :],
                    op[64:65, 0:196])

        def emit_output(b):
            nc.sync.dma_start(
                den2[b],
                den[b].rearrange("(a c) d e -> a c d e", c=32)[:, 0, :, :])
            nc.vector.reciprocal(recs[b], den2[b])
            hsw = hswp.tile([128, 8, NQ], BF16, tag="hsw", name="hsw")
            for t in range(8):
                rep = mmp.tile([128, 512], F32, tag="mm", name="rep")
                if t < 4:
                    # K=8 slice (heads 0-7 cover all nonzero sel rows here);
                    # also probes small-K matmul behavior
                    nc.tensor.matmul(rep[:, :NQ], lhsT=sel[0:8, t, :],
                                     rhs=recs[b][0:8, :],
                                     start=True, stop=True)
                else:
                    nc.tensor.matmul(rep[:, :NQ], lhsT=sel[:, t, :],
                                     rhs=recs[b], start=True, stop=True)
                t1 = tmpp.tile([128, NQ], BF16, tag="t1", name="t1")
                nc.vector.tensor_tensor(t1, acc[b][:, t, :], rep[:, :NQ],
                                        ALU.mult)
                vv = tmpp.tile([128, NQ], BF16, tag="vv", name="vv")
                nc.scalar.activation(vv, t1, AF.Identity,
                                     bias=shv_sb[:, t:t + 1])
                t3 = tmpp.tile([128, NQ], BF16, tag="t3", name="t3")
                nc.vector.tensor_scalar(t3, vv, -3.0, 3.0, ALU.max, ALU.min)
                nc.vector.scalar_tensor_tensor(hsw[:, t, :], t3, 3.0, vv,
                                               ALU.add, ALU.mult)
            for mt, msz in ((0, 128), (1, 68)):
                po = mmp.tile([128, 512], F32, tag="mm", name="po")
                nc.tensor.matmul(po[:msz, :],
                                 lhsT=ones1.bitcast(F32R)[0:1, 0:msz],
                                 rhs=shp_sb, start=True, stop=False,
                                 skip_group_check=True)
                for kk in range(8):
                    nc.tensor.matmul(
                        po[:msz, :],
                        lhsT=hsw[:, kk, 128 * mt:128 * mt + msz],
                        rhs=wp_sb[:, kk, :], start=False,
                        stop=(kk == 7), skip_group_check=True)
                fin = finp.tile([128, OUT], F32, tag="fin", name="fin")
                nc.scalar.activation(fin[:msz, :], po[:msz, :], AF.Copy)
                nc.sync.dma_start(out[b, 128 * mt:128 * mt + msz, :],
                                  fin[:msz, :])

        def emit_phaseA(g, b):
            # ---- phase A: k, q, v for one (group, batch elem) ----
            xtb = xp.tile([128, 3, N], F32R, tag="xtb", name="xtb")
            nc.sync.dma_start(xtb, xT[b].rearrange("(c p) n -> p c n", p=128))
            xstb = xp.tile([128, 3, NQ], BF16, tag="xstb", name="xstb")
            nc.sync.dma_start(xstb, xsT[b].rearrange("(c p) n -> p c n", p=128))

            # k for this head group: features [256g, 256g+256), feat-major
            for m2 in range(2):
                for n2 in range(2):
                    pk = mmp.tile([128, 512], F32, tag="mm", name="pk")
                    for kk in range(3):
                        nc.tensor.matmul(
                            pk[:, :392],
                            lhsT=wk_sb[:, kk, 256 * g + 128 * m2:
                                       256 * g + 128 * m2 + 128],
                            rhs=xtb[:, kk, 392 * n2:392 * n2 + 392],
                            start=(kk == 0), stop=(kk == 2))
                    nc.vector.tensor_scalar_add(
                        kg[b][:, m2, 392 * n2:392 * n2 + 392],
                        pk[:, :392],
                        shk_sb[:, 2 * g + m2:2 * g + m2 + 1])

            if g == 0 and b == 0:
                # deferred so the first xtb load outruns it on the DMA queue
                nc.sync.dma_start(wq_sb,
                                  wq.rearrange("(c p) n -> p c n", p=128))

            # q for this head group (bf16 matmul)
            for m2 in range(2):
                pq = mmp.tile([128, 512], F32, tag="mm", name="pq")
                for kk in range(3):
                    nc.tensor.matmul(
                        pq[:, :NQ],
                        lhsT=wq_sb[:, kk, 256 * g + 128 * m2:
                                   256 * g + 128 * m2 + 128],
                        rhs=xstb[:, kk, :],
                        start=(kk == 0), stop=(kk == 2))
                nc.vector.tensor_scalar_add(
                    qg[b][:, m2, :], pq[:, :NQ],
                    shq_sb[:, 2 * g + m2:2 * g + m2 + 1])

            if g == 0 and b == 0:
                nc.sync.dma_start(wv_sb,
                                  wv.rearrange("(c p) n -> p c n", p=128))

            # v token-major for this head group (512 features), with an
            # all-ones column appended per head for the softmax denominator
            if g == 0:
                ones_cols = vt[b].rearrange(
                    "p c (h e) -> p c h e", e=65)[:, :, :, 64:65]
                nc.vector.memset(ones_cols, 1.0)
            for c in range(C):
                pv = mmp.tile([128, 512], F32, tag="mm", name="pv")
                for kk in range(3):
                    nc.tensor.matmul(
                        pv[:MC, :],
                        lhsT=xtb[:, kk, MC * c:MC * c + MC],
                        rhs=wv_sb[:, kk, 512 * g:512 * g + 512],
                        start=(kk == 0), stop=(kk == 2))
                nc.vector.tensor_copy(
                    vt[b].rearrange("p c (h e) -> p c h e", e=65)[:, c, :, 0:64],
                    pv[:MC, :].rearrange("p (h d) -> p h d", d=64))

        def emit_late_weights():
            nc.sync.dma_start(wp_sb, wp.rearrange("(c p) n -> p c n", p=128))
            nc.sync.dma_start(shp_sb, shp)

        emitted_A = set()
        for g in range(G):
            # flush the cross-group pending attn before phase A overwrites
            # the vt tiles it reads
            if pending is not None:
                pb_, phh, pbatch, pg = pending
                emit_attn(pb_, phh, pbatch, pg)
                if pg == 0 and phh[1] == HG - 1 and (1, pbatch) not in emitted_A:
                    emit_phaseA(1, pbatch)
                    emitted_A.add((1, pbatch))
                pending = None
            for b in range(BC):
                if (g, b) not in emitted_A:
                    emit_phaseA(g, b)
                    emitted_A.add((g, b))

            # ---- phase B: scores + attn, bias reused across batch ----
            for hp in range(HG // 2):
                hhs = (2 * hp, 2 * hp + 1)
                bias_g = biasp.tile([MC, 2, C, NQ], BF16, tag="bias")
                nc.sync.dma_start(
                    bias_g,
                    ebias[:, 8 * g + 2 * hp:8 * g + 2 * hp + 2, :, :])
                for b in range(BC):
                    texp2 = [texpp.tile([MC, C, NQ], BF16, tag="texp",
                                        name=f"texp{j}") for j in range(2)]
                    # scores in 2-bank tiles (4 chunks packed as 2x392);
                    # exps batched per cq half to amortize the Act engine's
                    # fixed per-instruction cost (Act paces this phase)
                    for cq, cs in ((0, (0, 1, 2, 3)), (1, (4, 5, 6))):
                        for j, hh in enumerate(hhs):
                            pb = 32 * (hh % 4)
                            m2 = hh // 4
                            sc = scp.tile([MC, 2, 512], F32, tag="sc",
                                          name="sc")
                            for ci, c in enumerate(cs):
                                nc.tensor.matmul(
                                    sc[:, ci // 2,
                                       196 * (ci % 2):196 * (ci % 2) + 196],
                                    lhsT=kg[b][pb:pb + 32, m2, MC * c:MC * c + MC],
                                    rhs=qg[b][pb:pb + 32, m2, :],
                                    start=True, stop=True,
                                    tile_position=(pb, 0),
                                    skip_group_check=True)
                            eng = nc.gpsimd if j == 0 else nc.vector
                            if cq == 0:
                                nc.scalar.activation(
                                    texp2[j][:, 0:4, :].rearrange(
                                        "p (a b) q -> p a b q", b=2),
                                    sc[:, :, 0:392].rearrange(
                                        "p a (b q) -> p a b q", q=196), AF.Exp)
                                eng.tensor_tensor(
                                    texp2[j][:, 0:4, :], texp2[j][:, 0:4, :],
                                    bias_g[:, j, 0:4, :], ALU.mult)
                            else:
                                nc.scalar.activation(
                                    texp2[j][:, 4:6, :],
                                    sc[:, 0, 0:392].rearrange(
                                        "p (a q) -> p a q", q=196), AF.Exp)
                                nc.scalar.activation(texp2[j][:, 6, :],
                                                     sc[:, 1, 0:196], AF.Exp)
                                eng.tensor_tensor(
                                    texp2[j][:, 4:7, :], texp2[j][:, 4:7, :],
                                    bias_g[:, j, 4:7, :], ALU.mult)

                    # attn @ v for the PREVIOUS iteration: by the time PE
                    # reaches these matmuls, that iteration's exp+bias chain
                    # has had a full iteration of slack, so PE never stalls
                    # on texp readiness.
                    if pending is not None:
                        pb_, phh, pbatch, pg = pending
                        emit_attn(pb_, phh, pbatch, pg)
                        if phh[1] == HG - 1:
                            if pg == G - 1:
                                # last head pair of pbatch done: its output
                                # projection interleaves with remaining
                                # attention iterations
                                emit_output(pbatch)
                            elif (1, pbatch) not in emitted_A:
                                # pbatch finished group 0: its group-1 k/q/v
                                # compute and the first-half output chain
                                # interleave with remaining group-0 attention
                                emit_phaseA(1, pbatch)
                                emitted_A.add((1, pbatch))
                                if pbatch == 0:
                                    emit_late_weights()
                    pending = (texp2, hhs, b, g)

            if g == G - 1:
                pb_, phh, pbatch, pg = pending
                emit_attn(pb_, phh, pbatch, pg)
                emit_output(pbatch)
                pending = None
    nc.compile()
    return nc


def _prepare_in_maps(inputs):
    inp = {k: np.asarray(v) for k, v in inputs.items()}
    x = inp["x"].astype(np.float32)          # [32, 784, 384]
    Wkv, Wq, Wp = inp["Wkv"], inp["Wq"], inp["Wp"]
    biases, idxs = inp["biases"], inp["idxs"].astype(np.int64)

    s_kv = inp["kv_w"] / np.sqrt(inp["kv_var"] + EPS)
    wkv = (Wkv * s_kv[:, None]).astype(np.float32)
    sh_kv = (inp["kv_b"] - inp["kv_mean"] * s_kv).astype(np.float32)
    wkv3 = wkv.reshape(H, KD + D, IN)
    sh3 = sh_kv.reshape(H, KD + D)
    wkT = np.ascontiguousarray(wkv3[:, :KD, :].reshape(H * KD, IN).T)
    sh_k = np.ascontiguousarray(sh3[:, :KD].reshape(H * KD))
    wvT = np.ascontiguousarray(wkv3[:, KD:, :].reshape(H * D, IN).T)
    sh_v = np.ascontiguousarray(sh3[:, KD:].reshape(H * D))

    s_q = inp["q_w"] / np.sqrt(inp["q_var"] + EPS)
    wqT = np.ascontiguousarray(
        (Wq * (s_q * SCALE)[:, None]).T.astype(ml_dtypes.bfloat16))
    sh_q = ((inp["q_b"] - inp["q_mean"] * s_q) * SCALE).astype(np.float32)

    s_p = inp["p_w"] / np.sqrt(inp["p_var"] + EPS)
    wpT = np.ascontiguousarray(
        ((Wp * s_p[:, None]) / 6.0).T.astype(ml_dtypes.bfloat16))
    sh_p = (inp["p_b"] - inp["p_mean"] * s_p).astype(np.float32)

    eb = np.exp(biases.astype(np.float64))[:, idxs]      # [16, 196, 784]
    eb = eb.transpose(0, 2, 1).reshape(H, C, MC, NQ)
    eb = np.ascontiguousarray(eb.transpose(2, 0, 1, 3)).astype(ml_dtypes.bfloat16)

    xs = x.reshape(B, RES, RES, IN)[:, ::STRIDE, ::STRIDE].reshape(B, NQ, IN)

    shk_h = np.ascontiguousarray(sh_k.reshape(4, 128).T)
    shq_h = np.ascontiguousarray(sh_q.reshape(4, 128).T)
    shv_h = np.ascontiguousarray(sh_v.reshape(8, 128).T)
    shp_h = np.ascontiguousarray(sh_p.reshape(1, OUT))

    sel_h = np.zeros((16, 8, 128), ml_dtypes.bfloat16)
    for t in range(8):
        sel_h[2 * t, t, 0:64] = 1.0
        sel_h[2 * t + 1, t, 64:128] = 1.0
    shared = {"wk": wkT, "wv": wvT, "wq": wqT, "wp": wpT, "shk": shk_h,
              "shq": shq_h, "shv": shv_h, "shp": shp_h, "ebias": eb,
              "seld": sel_h}
    in_maps = []
    for i in range(NCORES):
        xb = x[BC * i:BC * i + BC]
        xsb = xs[BC * i:BC * i + BC]
        m = dict(shared)
        m["xT"] = np.ascontiguousarray(xb.transpose(0, 2, 1))
        m["xsT"] = np.ascontiguousarray(
            xsb.transpose(0, 2, 1).astype(ml_dtypes.bfloat16))
        in_maps.append(m)
    return in_maps


def kernel(**inputs):
    global _NC_CACHE, LAST_RESULTS
    in_maps = _prepare_in_maps(inputs)
    if _NC_CACHE is None:
        _NC_CACHE = _build_nc()
    res = run_bass_kernel_spmd(_NC_CACHE, in_maps,
                               core_ids=list(range(NCORES)), trace=TRACE)
    LAST_RESULTS = res
    return np.concatenate([res.results[i]["out"] for i in range(NCORES)],
                          axis=0)


# revision 49
# speedup vs baseline: 1.0365x; 1.0096x over previous
"""AttentionSubsample Trainium2 kernel: 8-core data-parallel over batch.

Layout strategy (per core, 4 batch elements):
  - All matmuls contract over the SBUF partition dim.
  - k (fp32r matmul) and q (bf16 matmul) are evicted to bf16 with the BN
    shift added per-partition; scores then run as bf16 matmuls at full PE
    rate regardless of the 196-wide moving dim.
  - v computed token-major fp32r, evicted to bf16 [tok, head-blocks of
    64 dims + a ones column] so attn@v needs no transpose and yields the
    softmax denominator as output row 64 for free.
  - exp'd scores are written bf16 and multiplied by a host-gathered bf16
    exp-bias table (exp(s)*exp(bias)); the multiply is split between the
    DVE (2x bf16 mode) and the otherwise-idle Pool engine.
  - the bias table is loaded once per (head-pair, group) and reused across
    all 4 batch elements (b is the innermost loop of the attention phase).
  - denominator rows are staged at 32-aligned partitions (the partition
    crossbar only allows 32-aligned shifts), DMA-gathered, reciprocal'd to
    bf16, and broadcast back over 64-row blocks with a tiny bf16 matmul.
  - hardswish runs on the Pool engine (SBUF-only) in bf16; the projection
    is fp32r with the BN shift injected via a K=1 ones-row matmul.
"""

import sys

sys.path.insert(0, "/opt/trn_rl_repo")

from contextlib import ExitStack

import numpy as np
import ml_dtypes

import concourse.bass as bass
import concourse.tile as tile
from concourse import bacc
from concourse import mybir
from concourse.bass_utils import run_bass_kernel_spmd

F32 = mybir.dt.float32
F32R = mybir.dt.float32r
BF16 = mybir.dt.bfloat16
ALU = mybir.AluOpType
AF = mybir.ActivationFunctionType

B, N, NQ, IN, H, KD, D, OUT = 32, 784, 196, 384, 16, 32, 64, 512
HID, DH = 1536, 1024
RES, RES_, STRIDE = 28, 14, 2
SCALE = KD ** -0.5
EPS = 1e-5
NCORES = 8
BC = B // NCORES          # 4 batch elems per core
C, MC = 7, 112            # key-token chunks: 7 x 112 = 784
G, HG = 2, 8              # 2 head-groups of 8 heads

TRACE = False
LAST_RESULTS = None

_NC_CACHE = None


def _build_nc():
    nc = bacc.Bacc("TRN2", target_bir_lowering=False, debug=False,
                   num_devices=NCORES)

    xT = nc.dram_tensor("xT", [BC, IN, N], BF16, kind="ExternalInput").ap()
    xsT = nc.dram_tensor("xsT", [BC, IN, NQ], BF16, kind="ExternalInput").ap()
    wk = nc.dram_tensor("wk", [IN, 512], BF16, kind="ExternalInput").ap()
    wv = nc.dram_tensor("wv", [IN, DH], BF16, kind="ExternalInput").ap()
    wq = nc.dram_tensor("wq", [IN, 512], BF16, kind="ExternalInput").ap()
    wp = nc.dram_tensor("wp", [DH, OUT], BF16, kind="ExternalInput").ap()
    shk = nc.dram_tensor("shk", [128, 4], F32, kind="ExternalInput").ap()
    shq = nc.dram_tensor("shq", [128, 4], F32, kind="ExternalInput").ap()
    shv = nc.dram_tensor("shv", [128, 8], F32, kind="ExternalInput").ap()
    shp = nc.dram_tensor("shp", [1, OUT], F32R, kind="ExternalInput").ap()
    ebias = nc.dram_tensor("ebias", [MC, H, C, NQ], BF16,
                           kind="ExternalInput").ap()
    seld = nc.dram_tensor("seld", [16, 8, 128], BF16, kind="ExternalInput").ap()
    out = nc.dram_tensor("out", [BC, NQ, OUT], F32, kind="ExternalOutput").ap()




    with tile.TileContext(nc) as tc, ExitStack() as ctx:
        ctx.enter_context(nc.allow_low_precision(
            reason="bf16 attention path validated against fp32 reference"))
        singles = ctx.enter_context(tc.tile_pool(name="singles", bufs=1))
        biasp = ctx.enter_context(tc.tile_pool(name="biasp", bufs=3))
        xp = ctx.enter_context(tc.tile_pool(name="xp", bufs=3))
        texpp = ctx.enter_context(tc.tile_pool(name="texpp", bufs=8))
        tmpp = ctx.enter_context(tc.tile_pool(name="tmpp", bufs=2))
        hswp = ctx.enter_context(tc.tile_pool(name="hswp", bufs=2))
        finp = ctx.enter_context(tc.tile_pool(name="finp", bufs=2))
        mmp = ctx.enter_context(tc.tile_pool(name="mmp", bufs=2, space="PSUM"))
        scp = ctx.enter_context(tc.tile_pool(name="scp", bufs=2, space="PSUM"))
        opp = ctx.enter_context(tc.tile_pool(name="opp", bufs=2, space="PSUM"))

        # --- persistent SBUF ---
        wk_sb = singles.tile([128, 3, 512], BF16)
        nc.sync.dma_start(wk_sb, wk.rearrange("(c p) n -> p c n", p=128))
        wq_sb = singles.tile([128, 3, 512], BF16)
        wv_sb = singles.tile([128, 3, DH], BF16)
        wp_sb = singles.tile([128, 8, OUT], BF16)
        shk_sb = singles.tile([128, 4], F32)
        nc.sync.dma_start(shk_sb, shk)
        shq_sb = singles.tile([128, 4], F32)
        nc.sync.dma_start(shq_sb, shq)
        shv_sb = singles.tile([128, 8], F32)
        nc.sync.dma_start(shv_sb, shv)
        shp_sb = singles.tile([1, OUT], F32R)
        # sel[:, t, :] is a [16, 128] 0/1 matrix: sel[i, t, m] = 1 iff head i
        # feeds output row m of feature-tile t (rows 0-63 <- head 2t, 64-127
        # <- head 2t+1). Used to broadcast softmax reciprocals across rows.
        sel = singles.tile([16, 8, 128], BF16)
        nc.sync.dma_start(sel, seld)
        ones1 = singles.tile([1, 128], F32)
        nc.gpsimd.memset(ones1, 1.0)

        acc = [singles.tile([128, 8, NQ], BF16, name=f"acc{b}")
               for b in range(BC)]
        # denominator staging: head h=4g+hh -> partition 32*hh, block g
        den = [singles.tile([128, 4, NQ], F32, name=f"den{b}")
               for b in range(BC)]
        den2 = [singles.tile([16, NQ], F32, name=f"den2{b}") for b in range(BC)]
        recs = [singles.tile([16, NQ], BF16, name=f"rec{b}") for b in range(BC)]

        # per-b working tensors for the current head group
        kg = [singles.tile([128, 2, N], BF16, name=f"kg{b}") for b in range(BC)]
        qg = [singles.tile([128, 2, NQ], BF16, name=f"qg{b}")
              for b in range(BC)]
        vt = [singles.tile([MC, C, 8 * 65], BF16, name=f"vt{b}")
              for b in range(BC)]

        pending = None

        def emit_attn(texp2, hhs, b, g):
            # j=1 first: its bias product runs on the faster DVE path, so it
            # is ready sooner; j=0 (Pool) gets extra slack.
            for j, hh in ((1, hhs[1]), (0, hhs[0])):
                h = 8 * g + hh
                op = opp.tile([65, 256], F32, tag="op", name="op")
                for c in range(C):
                    nc.tensor.matmul(op[:, 0:196],
                                     lhsT=vt[b][:, c, 65 * hh:65 * hh + 65],
                                     rhs=texp2[j][:, c, :],
                                     start=(c == 0), stop=(c == C - 1))
                t = h // 2
                r0 = 64 * (h % 2)
                nc.vector.tensor_copy(acc[b][r0:r0 + 64, t, :],
                                      op[0:64, 0:196])
                nc.vector.tensor_copy(
                    den[b][32 * (h // 4):32 * (h // 4) + 1, h % 4, :],
                    op[64:65, 0:196])

        def emit_output(b):
            nc.sync.dma_start(
                den2[b],
                den[b].rearrange("(a c) d e -> a c d e", c=32)[:, 0, :, :])
            nc.vector.reciprocal(recs[b], den2[b])
            hsw = hswp.tile([128, 8, NQ], BF16, tag="hsw", name="hsw")
            for t in range(8):
                rep = mmp.tile([128, 512], F32, tag="mm", name="rep")
                if t < 4:
                    # K=8 slice (heads 0-7 cover all nonzero sel rows here);
                    # also probes small-K matmul behavior
                    nc.tensor.matmul(rep[:, :NQ], lhsT=sel[0:8, t, :],
                                     rhs=recs[b][0:8, :],
                                     start=True, stop=True)
                else:
                    nc.tensor.matmul(rep[:, :NQ], lhsT=sel[:, t, :],
                                     rhs=recs[b], start=True, stop=True)
                t1 = tmpp.tile([128, NQ], BF16, tag="t1", name="t1")
                nc.vector.tensor_tensor(t1, acc[b][:, t, :], rep[:, :NQ],
                                        ALU.mult)
                vv = tmpp.tile([128, NQ], BF16, tag="vv", name="vv")
                nc.scalar.activation(vv, t1, AF.Identity,
                                     bias=shv_sb[:, t:t + 1])
                t3 = tmpp.tile([128, NQ], BF16, tag="t3", name="t3")
                nc.vector.tensor_scalar(t3, vv, -3.0, 3.0, ALU.max, ALU.min)
                nc.vector.scalar_tensor_tensor(hsw[:, t, :], t3, 3.0, vv,
                                               ALU.add, ALU.mult)
            for mt, msz in ((0, 128), (1, 68)):
                po = mmp.tile([128, 512], F32, tag="mm", name="po")
                nc.tensor.matmul(po[:msz, :],
                                 lhsT=ones1.bitcast(F32R)[0:1, 0:msz],
                                 rhs=shp_sb, start=True, stop=False,
                                 skip_group_check=True)
                for kk in range(8):
                    nc.tensor.matmul(
                        po[:msz, :],
                        lhsT=hsw[:, kk, 128 * mt:128 * mt + msz],
                        rhs=wp_sb[:, kk, :], start=False,
                        stop=(kk == 7), skip_group_check=True)
                fin = finp.tile([128, OUT], F32, tag="fin", name="fin")
                nc.scalar.activation(fin[:msz, :], po[:msz, :], AF.Copy)
                nc.sync.dma_start(out[b, 128 * mt:128 * mt + msz, :],
                                  fin[:msz, :])

        def emit_phaseA(g, b):
            # ---- phase A: k, q, v for one (group, batch elem) ----
            xtb = xp.tile([128, 3, N], BF16, tag="xtb", name="xtb")
            nc.sync.dma_start(xtb, xT[b].rearrange("(c p) n -> p c n", p=128))
            xstb = xp.tile([128, 3, NQ], BF16, tag="xstb", name="xstb")
            nc.sync.dma_start(xstb, xsT[b].rearrange("(c p) n -> p c n", p=128))

            # k for this head group: features [256g, 256g+256), feat-major
            for m2 in range(2):
                for n2 in range(2):
                    pk = mmp.tile([128, 512], F32, tag="mm", name="pk")
                    for kk in range(3):
                        nc.tensor.matmul(
                            pk[:, :392],
                            lhsT=wk_sb[:, kk, 256 * g + 128 * m2:
                                       256 * g + 128 * m2 + 128],
                            rhs=xtb[:, kk, 392 * n2:392 * n2 + 392],
                            start=(kk == 0), stop=(kk == 2))
                    nc.vector.tensor_scalar_add(
                        kg[b][:, m2, 392 * n2:392 * n2 + 392],
                        pk[:, :392],
                        shk_sb[:, 2 * g + m2:2 * g + m2 + 1])

            if g == 0 and b == 0:
                # deferred so the first xtb load outruns it on the DMA queue
                nc.sync.dma_start(wq_sb,
                                  wq.rearrange("(c p) n -> p c n", p=128))

            # q for this head group (bf16 matmul)
            for m2 in range(2):
                pq = mmp.tile([128, 512], F32, tag="mm", name="pq")
                for kk in range(3):
                    nc.tensor.matmul(
                        pq[:, :NQ],
                        lhsT=wq_sb[:, kk, 256 * g + 128 * m2:
                                   256 * g + 128 * m2 + 128],
                        rhs=xstb[:, kk, :],
                        start=(kk == 0), stop=(kk == 2))
                nc.vector.tensor_scalar_add(
                    qg[b][:, m2, :], pq[:, :NQ],
                    shq_sb[:, 2 * g + m2:2 * g + m2 + 1])

            if g == 0 and b == 0:
                nc.sync.dma_start(wv_sb,
                                  wv.rearrange("(c p) n -> p c n", p=128))

            # v token-major for this head group (512 features), with an
            # all-ones column appended per head for the softmax denominator
            if g == 0:
                ones_cols = vt[b].rearrange(
                    "p c (h e) -> p c h e", e=65)[:, :, :, 64:65]
                nc.vector.memset(ones_cols, 1.0)
            for c in range(C):
                pv = mmp.tile([128, 512], F32, tag="mm", name="pv")
                for kk in range(3):
                    nc.tensor.matmul(
                        pv[:MC, :],
                        lhsT=xtb[:, kk, MC * c:MC * c + MC],
                        rhs=wv_sb[:, kk, 512 * g:512 * g + 512],
                        start=(kk == 0), stop=(kk == 2))
                nc.vector.tensor_copy(
                    vt[b].rearrange("p c (h e) -> p c h e", e=65)[:, c, :, 0:64],
                    pv[:MC, :].rearrange("p (h d) -> p h d", d=64))

        def emit_late_weights():
            nc.sync.dma_start(wp_sb, wp.rearrange("(c p) n -> p c n", p=128))
            nc.sync.dma_start(shp_sb, shp)

        emitted_A = set()
        for g in range(G):
            # flush the cross-group pending attn before phase A overwrites
            # the vt tiles it reads
            if pending is not None:
                pb_, phh, pbatch, pg = pending
                emit_attn(pb_, phh, pbatch, pg)
                if pg == 0 and phh[1] == HG - 1 and (1, pbatch) not in emitted_A:
                    emit_phaseA(1, pbatch)
                    emitted_A.add((1, pbatch))
                pending = None
            for b in range(BC):
                if (g, b) not in emitted_A:
                    emit_phaseA(g, b)
                    emitted_A.add((g, b))

            # ---- phase B: scores + attn, bias reused across batch ----
            for hp in range(HG // 2):
                hhs = (2 * hp, 2 * hp + 1)
                bias_g = biasp.tile([MC, 2, C, NQ], BF16, tag="bias")
                nc.sync.dma_start(
                    bias_g,
                    ebias[:, 8 * g + 2 * hp:8 * g + 2 * hp + 2, :, :])
                for b in range(BC):
                    texp2 = [texpp.tile([MC, C, NQ], BF16, tag="texp",
                                        name=f"texp{j}") for j in range(2)]
                    # scores in 2-bank tiles (4 chunks packed as 2x392);
                    # exps batched per cq half to amortize the Act engine's
                    # fixed per-instruction cost (Act paces this phase)
                    for cq, cs in ((0, (0, 1, 2, 3)), (1, (4, 5, 6))):
                        for j, hh in enumerate(hhs):
                            pb = 32 * (hh % 4)
                            m2 = hh // 4
                            sc = scp.tile([MC, 2, 512], F32, tag="sc",
                                          name="sc")
                            for ci, c in enumerate(cs):
                                nc.tensor.matmul(
                                    sc[:, ci // 2,
                                       196 * (ci % 2):196 * (ci % 2) + 196],
                                    lhsT=kg[b][pb:pb + 32, m2, MC * c:MC * c + MC],
                                    rhs=qg[b][pb:pb + 32, m2, :],
                                    start=True, stop=True,
                                    tile_position=(pb, 0),
                                    skip_group_check=True)
                            eng = nc.gpsimd if j == 0 else nc.vector
                            if cq == 0:
                                nc.scalar.activation(
                                    texp2[j][:, 0:4, :].rearrange(
                                        "p (a b) q -> p a b q", b=2),
                                    sc[:, :, 0:392].rearrange(
                                        "p a (b q) -> p a b q", q=196), AF.Exp)
                                eng.tensor_tensor(
                                    texp2[j][:, 0:4, :], texp2[j][:, 0:4, :],
                                    bias_g[:, j, 0:4, :], ALU.mult)
                            else:
                                nc.scalar.activation(
                                    texp2[j][:, 4:6, :],
                                    sc[:, 0, 0:392].rearrange(
                                        "p (a q) -> p a q", q=196), AF.Exp)
                                nc.scalar.activation(texp2[j][:, 6, :],
                                                     sc[:, 1, 0:196], AF.Exp)
                                eng.tensor_tensor(
                                    texp2[j][:, 4:7, :], texp2[j][:, 4:7, :],
                                    bias_g[:, j, 4:7, :], ALU.mult)

                    # attn @ v for the PREVIOUS iteration: by the time PE
                    # reaches these matmuls, that iteration's exp+bias chain
                    # has had a full iteration of slack, so PE never stalls
                    # on texp readiness.
                    if pending is not None:
                        pb_, phh, pbatch, pg = pending
                        emit_attn(pb_, phh, pbatch, pg)
                        if phh[1] == HG - 1:
                            if pg == G - 1:
                                # last head pair of pbatch done: its output
                                # projection interleaves with remaining
                                # attention iterations
                                emit_output(pbatch)
                            elif (1, pbatch) not in emitted_A:
                                # pbatch finished group 0: its group-1 k/q/v
                                # compute and the first-half output chain
                                # interleave with remaining group-0 attention
                                emit_phaseA(1, pbatch)
                                emitted_A.add((1, pbatch))
                                if pbatch == 0:
                                    emit_late_weights()
                    pending = (texp2, hhs, b, g)

            if g == G - 1:
                pb_, phh, pbatch, pg = pending
                emit_attn(pb_, phh, pbatch, pg)
                emit_output(pbatch)
                pending = None
    nc.compile()
    return nc


def _prepare_in_maps(inputs):
    inp = {k: np.asarray(v) for k, v in inputs.items()}
    x = inp["x"].astype(np.float32)          # [32, 784, 384]
    Wkv, Wq, Wp = inp["Wkv"], inp["Wq"], inp["Wp"]
    biases, idxs = inp["biases"], inp["idxs"].astype(np.int64)

    s_kv = inp["kv_w"] / np.sqrt(inp["kv_var"] + EPS)
    wkv = (Wkv * s_kv[:, None]).astype(np.float32)
    sh_kv = (inp["kv_b"] - inp["kv_mean"] * s_kv).astype(np.float32)
    wkv3 = wkv.reshape(H, KD + D, IN)
    sh3 = sh_kv.reshape(H, KD + D)
    wkT = np.ascontiguousarray(
        wkv3[:, :KD, :].reshape(H * KD, IN).T.astype(ml_dtypes.bfloat16))
    sh_k = np.ascontiguousarray(sh3[:, :KD].reshape(H * KD))
    wvT = np.ascontiguousarray(
        wkv3[:, KD:, :].reshape(H * D, IN).T.astype(ml_dtypes.bfloat16))
    sh_v = np.ascontiguousarray(sh3[:, KD:].reshape(H * D))

    s_q = inp["q_w"] / np.sqrt(inp["q_var"] + EPS)
    wqT = np.ascontiguousarray(
        (Wq * (s_q * SCALE)[:, None]).T.astype(ml_dtypes.bfloat16))
    sh_q = ((inp["q_b"] - inp["q_mean"] * s_q) * SCALE).astype(np.float32)

    s_p = inp["p_w"] / np.sqrt(inp["p_var"] + EPS)
    wpT = np.ascontiguousarray(
        ((Wp * s_p[:, None]) / 6.0).T.astype(ml_dtypes.bfloat16))
    sh_p = (inp["p_b"] - inp["p_mean"] * s_p).astype(np.float32)

    eb = np.exp(biases.astype(np.float64))[:, idxs]      # [16, 196, 784]
    eb = eb.transpose(0, 2, 1).reshape(H, C, MC, NQ)
    eb = np.ascontiguousarray(eb.transpose(2, 0, 1, 3)).astype(ml_dtypes.bfloat16)

    xs = x.reshape(B, RES, RES, IN)[:, ::STRIDE, ::STRIDE].reshape(B, NQ, IN)

    shk_h = np.ascontiguousarray(sh_k.reshape(4, 128).T)
    shq_h = np.ascontiguousarray(sh_q.reshape(4, 128).T)
    shv_h = np.ascontiguousarray(sh_v.reshape(8, 128).T)
    shp_h = np.ascontiguousarray(sh_p.reshape(1, OUT))

    sel_h = np.zeros((16, 8, 128), ml_dtypes.bfloat16)
    for t in range(8):
        sel_h[2 * t, t, 0:64] = 1.0
        sel_h[2 * t + 1, t, 64:128] = 1.0
    shared = {"wk": wkT, "wv": wvT, "wq": wqT, "wp": wpT, "shk": shk_h,
              "shq": shq_h, "shv": shv_h, "shp": shp_h, "ebias": eb,
              "seld": sel_h}
    in_maps = []
    for i in range(NCORES):
        xb = x[BC * i:BC * i + BC]
        xsb = xs[BC * i:BC * i + BC]
        m = dict(shared)
        m["xT"] = np.ascontiguousarray(
            xb.transpose(0, 2, 1).astype(ml_dtypes.bfloat16))
        m["xsT"] = np.ascontiguousarray(
            xsb.transpose(0, 2, 1).astype(ml_dtypes.bfloat16))
        in_maps.append(m)
    return in_maps


def kernel(**inputs):
    global _NC_CACHE, LAST_RESULTS
    in_maps = _prepare_in_maps(inputs)
    if _NC_CACHE is None:
        _NC_CACHE = _build_nc()
    res = run_bass_kernel_spmd(_NC_CACHE, in_maps,
                               core_ids=list(range(NCORES)), trace=TRACE)
    LAST_RESULTS = res
    return np.concatenate([res.results[i]["out"] for i in range(NCORES)],
                          axis=0)
